# revision 20
# baseline (speedup 1.0000x reference)
"""SMEAR MoE layer (nn_MoELayer_SMEAR) Trainium2 Bass kernel.

Problem: B=8, L=2048, D=1024, H=4096, E=8, fp32.
  logits = x @ router_w.T + router_b; probs = softmax(logits) * mask
  up = probs.sum(L) / clip(mask.sum(L), 1)            # [B, E]
  mW1 = up @ W1 ; mb1 = up @ b1 ; mW2 = up @ W2 ; mb2 = up @ b2  (merged per b)
  out = relu(x @ mW1.T + mb1) @ mW2.T + mb2

Sharding (8 cores): dp=2 over B x tp=4 over H.
  core c: group g=c//4 handles batches g*4..g*4+3; rank r=c%4 handles
  H-shard [r*1024,(r+1)*1024). Each core computes partial outputs for its
  4 batches over its H-shard; host sums the 4 partials per group and
  transposes ([o,t] -> [t,o]) to unshard.

Device phases per core:
  B) router, transpose-free: logits^T [e,t] per 512 chunk (PE bf16), exp
     with +bias (ACT), per-token Z via ones8 matmul (replicated on 8
     partitions), 1/Z * mask * exp reduced over t on DVE -> up[8,b].
  C) merge via block-diagonal matmul: stationary S[128,64] f32r holds
     upT[8,4] in 16 g-blocks (S[(g,e),(b,g)] = up[b,e]); moving = f32r W
     chunks laid out [.., (g,e)=128, free]. One pass of W merges ALL 4
     batches at once (~27us PE per tensor). PSUM [64,512] -> SBUF stage
     (mW1: bf16, mW2: f32r) -> DRAM, read back per batch as [128, sub, n]
     stationary tiles.
  D) MLP per b: hiddenT = relu(mW1^T x^T + b): bf16 stationary x bf16
     moving, hid kept f32r; outT = mW2^T hiddenT + b2 all-f32r (owner core
     adds b2); partials to DRAM. W2 merge is emitted inside b0's shadow.
"""

import numpy as np
import ml_dtypes

import concourse.bass as bass
import concourse.bacc as bacc
import concourse.mybir as mybir
import concourse.tile as tile
from concourse.bass_utils import run_bass_kernel_spmd
from concourse.masks import make_identity

P = 128
B, L, D, H, E = 8, 2048, 1024, 4096, 8
NB = 4          # batches per core
HS = H // 4     # h-shard width per core
DS = D // P     # 8 d-subtiles
HSUB = HS // P  # 8 h-subtiles in shard
OSUB = D // P   # 8 output subtiles
TCH = 512       # moving-dim chunk for matmuls
TC = L // TCH   # 4 chunks per batch

F32 = mybir.dt.float32
F32R = mybir.dt.float32r
BF16 = mybir.dt.bfloat16
AF = mybir.ActivationFunctionType
ALU = mybir.AluOpType
AX = mybir.AxisListType

_CACHED_NC = None


def _build():
    nc = bacc.Bacc("TRN2", target_bir_lowering=False, debug=False)

    xTb = nc.dram_tensor("xTb", [NB, D, L], BF16, kind="ExternalInput")
    maskT = nc.dram_tensor("maskT", [L, NB], F32, kind="ExternalInput")
    maskR8 = nc.dram_tensor("maskR8", [NB, E, L], F32, kind="ExternalInput")
    rwT = nc.dram_tensor("rwT", [D, E], BF16, kind="ExternalInput")
    rb = nc.dram_tensor("rb", [E, 1], F32, kind="ExternalInput")
    # W1b[sb, dc, (g,e), h] = W1T_shard[e, sb*128 + dc*16 + g, h]  (f32r)
    W1b = nc.dram_tensor("W1b", [DS, 8, P, HS], F32R, kind="ExternalInput")
    # W2b[sb, hc, (g,e), o] = W2T_shard[e, sb*128 + hc*16 + g, o]  (f32r)
    W2b = nc.dram_tensor("W2b", [HSUB, 8, P, D], F32R, kind="ExternalInput")
    b1T = nc.dram_tensor("b1T", [HS, E], F32, kind="ExternalInput")
    b2T = nc.dram_tensor("b2T", [D, E], F32, kind="ExternalInput")
    ownc = nc.dram_tensor("ownc", [NB, 1], F32, kind="ExternalInput")
    gmaskc = nc.dram_tensor("gmaskc", [P, 16], F32, kind="ExternalInput")
    emapc = nc.dram_tensor("emapc", [P, E], F32, kind="ExternalInput")
    outp = nc.dram_tensor("outp", [NB, D, L], F32, kind="ExternalOutput")

    # merged weights round trip: [b, sb, c, g, free]
    mW1d = nc.dram_tensor("mW1d", [NB, DS, 8, 16, HS], BF16)
    mW2d = nc.dram_tensor("mW2d", [NB, HSUB, 8, 16, D], F32R)

    with tile.TileContext(nc) as tc:
        with tc.tile_pool(name="const", bufs=1) as const:
            ident = const.tile([P, P], F32)
            make_identity(nc, ident)
            ones_col = const.tile([P, 1], F32)
            nc.gpsimd.memset(ones_col[:], 1.0)
            ones_row = const.tile([1, P], F32)
            nc.gpsimd.memset(ones_row[:], 1.0)
            ones8f = const.tile([E, E], F32)
            nc.gpsimd.memset(ones8f[:], 1.0)
            ones8 = const.tile([E, E], F32R)
            nc.vector.tensor_copy(ones8[:], ones8f[:])

            rwT_sb = const.tile([P, DS, E], BF16)
            nc.sync.dma_start(rwT_sb[:], rwT.ap().rearrange("(s p) e -> p s e", p=P))
            rb_sb = const.tile([E, 1], F32)
            nc.sync.dma_start(rb_sb[:], rb.ap())
            maskT_sb = const.tile([P, L // P, NB], F32)
            nc.sync.dma_start(maskT_sb[:], maskT.ap().rearrange("(q p) b -> p q b", p=P))
            b1T_sb = const.tile([P, HSUB, E], F32)
            nc.sync.dma_start(b1T_sb[:], b1T.ap().rearrange("(s p) e -> p s e", p=P))
            b2T_sb = const.tile([P, OSUB, E], F32)
            nc.sync.dma_start(b2T_sb[:], b2T.ap().rearrange("(s p) e -> p s e", p=P))
            own_sb = const.tile([NB, 1], F32)
            nc.sync.dma_start(own_sb[:], ownc.ap())
            gmask_sb = const.tile([P, 16], F32)
            nc.sync.dma_start(gmask_sb[:], gmaskc.ap())
            emap_sb = const.tile([P, E], F32)
            nc.sync.dma_start(emap_sb[:], emapc.ap())

            up_sb = const.tile([E, NB], F32)
            upT_sb = const.tile([NB, E], F32)
            upTo_sb = const.tile([NB, E], F32)
            up_bc = const.tile([P, NB, E], F32)
            upo_bc = const.tile([P, NB, E], F32)
            mb1_sb = const.tile([P, NB, HSUB], F32)
            mb2_sb = const.tile([P, NB, OSUB], F32)
            invbc_sb = const.tile([P, NB], F32)
            S2_sb = const.tile([P, NB * 16], F32R)
            upRep = const.tile([P, NB], F32)
            tmp8 = const.tile([P, E], F32)

            # ---------------- Phase B: router ----------------
            with tc.tile_pool(name="rpsum", bufs=1, space="PSUM") as rpsum, \
                 tc.tile_pool(name="rsb", bufs=6) as rsb, \
                 tc.tile_pool(name="xrt", bufs=4) as xrt, \
                 tc.tile_pool(name="lgp", bufs=3, space="PSUM") as lgp, \
                 tc.tile_pool(name="dnp", bufs=2, space="PSUM") as dnp:

                # denominators: denom[b] = clip(sum_t mask, 1); invbc = 1/denom bcast
                mpart = rsb.tile([P, NB], F32)
                for b in range(NB):
                    nc.vector.tensor_reduce(
                        mpart[:, b:b + 1], maskT_sb[:, :, b], axis=AX.X, op=ALU.add)
                den_ps = rpsum.tile([NB, 1], F32, tag="rps")
                nc.tensor.matmul(den_ps[:], mpart[:], ones_col[:], start=True, stop=True)
                den_sb = rsb.tile([NB, 1], F32)
                nc.vector.tensor_scalar_max(den_sb[:], den_ps[:], 1.0)
                inv_sb = rsb.tile([NB, 1], F32)
                nc.vector.reciprocal(inv_sb[:], den_sb[:])
                invT_ps = rpsum.tile([1, NB], F32, tag="rps")
                nc.tensor.transpose(invT_ps[:], inv_sb[:], ident[:NB, :NB])
                invT_sb = rsb.tile([1, NB], F32)
                nc.vector.tensor_copy(invT_sb[:], invT_ps[:])
                invbc_ps = rpsum.tile([P, NB], F32, tag="rps")
                nc.tensor.matmul(invbc_ps[:], ones_row[:], invT_sb[:], start=True, stop=True)
                nc.vector.tensor_copy(invbc_sb[:], invbc_ps[:])

                # up[b] = invb * sum_t exp[e,t] * mask[t] / Z[t], computed in
                # [e,t] orientation (no transposes). The Z matmul for chunk j
                # is emitted after the logits matmuls of chunk j+1 so the PE
                # never waits on ACT exp.
                def den_stage(j, exs, m8, b):
                    dn_ps = dnp.tile([E, TCH], F32, tag="dn")
                    nc.tensor.matmul(dn_ps[:], ones8[:], exs[j][:],
                                     start=True, stop=True)
                    zm = rsb.tile([E, TCH], F32, tag="zm")
                    nc.vector.reciprocal(zm[:], dn_ps[:])
                    nc.vector.tensor_tensor(
                        zm[:], zm[:], m8[:, j * TCH:(j + 1) * TCH], ALU.mult)
                    t1 = rsb.tile([E, TCH], F32, tag="t1")
                    nc.vector.tensor_tensor(t1[:], exs[j][:], zm[:], ALU.mult)
                    red = rsb.tile([E, 1], F32, tag="red")
                    nc.vector.tensor_reduce(red[:], t1[:], axis=AX.X, op=ALU.add)
                    if j == 0:
                        nc.vector.tensor_copy(up_sb[:, b:b + 1], red[:])
                    else:
                        nc.vector.tensor_tensor(
                            up_sb[:, b:b + 1], up_sb[:, b:b + 1], red[:], ALU.add)

                for b in range(NB):
                    m8 = rsb.tile([E, L], F32, tag="m8")
                    nc.sync.dma_start(m8[:], maskR8.ap()[b])
                    exs = [None] * TC
                    for t4 in range(TC):
                        xt = xrt.tile([P, DS, TCH], BF16, tag="xrt")
                        nc.sync.dma_start(
                            xt[:],
                            xTb.ap()[b].rearrange("(s p) t -> p s t", p=P)[
                                :, :, t4 * TCH:(t4 + 1) * TCH])
                        lg_ps = lgp.tile([E, TCH], F32, tag="lg")
                        for dsb in range(DS):
                            nc.tensor.matmul(lg_ps[:], rwT_sb[:, dsb], xt[:, dsb],
                                             start=(dsb == 0), stop=(dsb == DS - 1))
                        ex = rsb.tile([E, TCH], F32R, tag="ex")
                        nc.scalar.activation(ex[:], lg_ps[:], AF.Exp, bias=rb_sb[:])
                        exs[t4] = ex
                        if t4 >= 1:
                            den_stage(t4 - 1, exs, m8, b)
                    den_stage(TC - 1, exs, m8, b)
                    nc.vector.tensor_scalar_mul(
                        up_sb[:, b:b + 1], up_sb[:, b:b + 1], invbc_sb[:E, b:b + 1])

                # broadcast up across partitions; owner-masked copy for b2
                upT_ps = rpsum.tile([NB, E], F32, tag="rps")
                nc.tensor.transpose(upT_ps[:], up_sb[:], ident[:E, :E])
                nc.vector.tensor_copy(upT_sb[:], upT_ps[:])
                nc.vector.tensor_scalar_mul(upTo_sb[:], upT_sb[:], own_sb[:])
                for b in range(NB):
                    rowu = rsb.tile([1, E], F32, tag="rowu")
                    nc.sync.dma_start(rowu[:], upT_sb[b:b + 1, :])
                    rowo = rsb.tile([1, E], F32, tag="rowo")
                    nc.sync.dma_start(rowo[:], upTo_sb[b:b + 1, :])
                    bc_ps = rpsum.tile([P, E], F32, tag="rps")
                    nc.tensor.matmul(bc_ps[:], ones_row[:], rowu[:], start=True, stop=True)
                    nc.vector.tensor_copy(up_bc[:, b], bc_ps[:])
                    bo_ps = rpsum.tile([P, E], F32, tag="rps")
                    nc.tensor.matmul(bo_ps[:], ones_row[:], rowo[:], start=True, stop=True)
                    nc.vector.tensor_copy(upo_bc[:, b], bo_ps[:])

                # merged biases: mb1[b] = sum_e up[b,e] b1T[:,e]; mb2 owner-masked
                for b in range(NB):
                    nc.vector.tensor_scalar_mul(
                        mb1_sb[:, b], b1T_sb[:, :, 0], up_bc[:, b, 0:1])
                    nc.vector.tensor_scalar_mul(
                        mb2_sb[:, b], b2T_sb[:, :, 0], upo_bc[:, b, 0:1])
                    for e in range(1, E):
                        nc.vector.scalar_tensor_tensor(
                            mb1_sb[:, b], b1T_sb[:, :, e], up_bc[:, b, e:e + 1],
                            mb1_sb[:, b], ALU.mult, ALU.add)
                        nc.vector.scalar_tensor_tensor(
                            mb2_sb[:, b], b2T_sb[:, :, e], upo_bc[:, b, e:e + 1],
                            mb2_sb[:, b], ALU.mult, ALU.add)

                # block-diagonal merge stationary S[(g,e), b*16+g] = up[b,e]
                for b in range(NB):
                    nc.vector.tensor_tensor(
                        tmp8[:], up_bc[:, b], emap_sb[:], ALU.mult)
                    nc.vector.tensor_reduce(
                        upRep[:, b:b + 1], tmp8[:], axis=AX.X, op=ALU.add)
                    nc.vector.tensor_scalar_mul(
                        S2_sb[:, b * 16:(b + 1) * 16], gmask_sb[:], upRep[:, b:b + 1])

            # ---------------- Phases C+D: merge + MLP ----------------
            with tc.tile_pool(name="mrgps", bufs=4, space="PSUM") as mrgps, \
                 tc.tile_pool(name="mwp", bufs=1) as mwp, \
                 tc.tile_pool(name="hidp", bufs=1) as hidp, \
                 tc.tile_pool(name="xtp", bufs=4) as xtp, \
                 tc.tile_pool(name="osbp", bufs=2) as osbp, \
                 tc.tile_pool(name="mmp", bufs=4, space="PSUM") as mmp:

                def emit_merge(Wb, mWd, dt_stage, nch, spc, wch, stgp,
                               dve_only=False):
                    """One f32r pass of W merges all 4 batches; a stage tile
                    spans spc chunk iterations, then one scatter per batch."""
                    for sb in range(8):
                        stg = None
                        for cq in range(8 // nch):
                            ch = wch.tile([P, nch, 1024], F32R, tag="wch")
                            nc.sync.dma_start(
                                ch[:],
                                Wb.ap()[sb, cq * nch:(cq + 1) * nch].rearrange(
                                    "c p h -> p c h"))
                            if cq % spc == 0:
                                stg = stgp.tile([64, spc * nch, 1024], dt_stage,
                                                tag="stg")
                            for c in range(nch):
                                cs = (cq % spc) * nch + c
                                for hh in range(2):
                                    ps = mrgps.tile([64, TCH], F32, tag="mps")
                                    nc.tensor.matmul(
                                        ps[:], S2_sb[:],
                                        ch[:, c, hh * TCH:(hh + 1) * TCH],
                                        start=True, stop=True)
                                    if dve_only or (cs + hh) % 2 == 0:
                                        nc.vector.tensor_copy(
                                            stg[:, cs, hh * TCH:(hh + 1) * TCH],
                                            ps[:])
                                    else:
                                        nc.scalar.activation(
                                            stg[:, cs, hh * TCH:(hh + 1) * TCH],
                                            ps[:], AF.Identity)
                            if cq % spc == spc - 1:
                                c0 = (cq - spc + 1) * nch
                                for b in range(NB):
                                    nc.sync.dma_start(
                                        mWd.ap()[b, sb, c0:c0 + spc * nch]
                                        .rearrange("c g h -> g c h"),
                                        stg[b * 16:(b + 1) * 16])

                with tc.tile_pool(name="w1ch", bufs=3) as w1ch, \
                     tc.tile_pool(name="stg1", bufs=2) as stg1p:
                    emit_merge(W1b, mW1d, BF16, 2, 2, w1ch, stg1p)
                # prefetch b0 merged weights with a long separation from
                # their consumers (the readback DMA completion is not a
                # reliable gate for immediately-following matmuls)
                mw1 = mwp.tile([P, DS, HS], BF16, tag="mw1")
                nc.sync.dma_start(
                    mw1[:], mW1d.ap()[0].rearrange("s c g h -> c g s h"))
                for b in range(NB):
                    hid = hidp.tile([P, HSUB, L], F32R, tag="hid")
                    for t4 in range(TC):
                        xt = xtp.tile([P, DS, TCH], BF16, tag="xt")
                        nc.sync.dma_start(
                            xt[:],
                            xTb.ap()[b].rearrange("(s p) t -> p s t", p=P)[
                                :, :, t4 * TCH:(t4 + 1) * TCH])
                        for hb in range(HSUB):
                            ps = mmp.tile([P, TCH], F32, tag="ps")
                            for dsb in range(DS):
                                nc.tensor.matmul(
                                    ps[:], mw1[:, dsb, hb * P:(hb + 1) * P],
                                    xt[:, dsb],
                                    start=(dsb == 0), stop=(dsb == DS - 1))
                            nc.scalar.activation(
                                hid[:, hb, t4 * TCH:(t4 + 1) * TCH], ps[:],
                                AF.Relu, bias=mb1_sb[:, b, hb:hb + 1])
                    if b + 1 < NB:
                        mw1n = mwp.tile([P, DS, HS], BF16, tag="mw1")
                        nc.sync.dma_start(
                            mw1n[:],
                            mW1d.ap()[b + 1].rearrange("s c g h -> c g s h"))
                    if b == 0:
                        with tc.tile_pool(name="w2ch", bufs=2) as w2ch, \
                             tc.tile_pool(name="stg2", bufs=2) as stg2p:
                            emit_merge(W2b, mW2d, F32R, 2, 2, w2ch, stg2p,
                                       dve_only=True)
                        mw2 = mwp.tile([P, HSUB, D], F32R, tag="mw2")
                        nc.sync.dma_start(
                            mw2[:],
                            mW2d.ap()[0].rearrange("s c g h -> c g s h"))
                        for t4 in (0, 1):
                            xt = xtp.tile([P, DS, TCH], BF16, tag="xt")
                            nc.sync.dma_start(
                                xt[:],
                                xTb.ap()[b].rearrange("(s p) t -> p s t", p=P)[
                                    :, :, t4 * TCH:(t4 + 1) * TCH])
                            for hb in range(HSUB):
                                ps = mmp.tile([P, TCH], F32, tag="ps")
                                for dsb in range(DS):
                                    nc.tensor.matmul(
                                        ps[:], mw1[:, dsb, hb * P:(hb + 1) * P],
                                        xt[:, dsb],
                                        start=(dsb == 0), stop=(dsb == DS - 1))
                                nc.scalar.activation(
                                    hid[:, hb, t4 * TCH:(t4 + 1) * TCH], ps[:],
                                    AF.Relu, bias=mb1_sb[:, b, hb:hb + 1])
                    for t4 in range(TC):
                        for obq in range(4):
                            ot = osbp.tile([P, 2, TCH], F32, tag="ot")
                            for oo in range(2):
                                ob = obq * 2 + oo
                                ps = mmp.tile([P, TCH], F32, tag="ps")
                                for hs in range(HSUB):
                                    nc.tensor.matmul(
                                        ps[:], mw2[:, hs, ob * P:(ob + 1) * P],
                                        hid[:, hs, t4 * TCH:(t4 + 1) * TCH],
                                        start=(hs == 0), stop=(hs == HSUB - 1))
                                nc.vector.tensor_scalar_add(
                                    ot[:, oo], ps[:], mb2_sb[:, b, ob:ob + 1])
                            nc.sync.dma_start(
                                outp.ap()[b, obq * 256:(obq + 1) * 256,
                                          t4 * TCH:(t4 + 1) * TCH].rearrange(
                                              "(o p) t -> p o t", p=P),
                                ot[:])
                    if b + 1 < NB:
                        mw2n = mwp.tile([P, HSUB, D], F32R, tag="mw2")
                        nc.sync.dma_start(
                            mw2n[:],
                            mW2d.ap()[b + 1].rearrange("s c g h -> c g s h"))
                        mw1, mw2 = mw1n, mw2n

    nc.compile()
    return nc


def _get_nc():
    global _CACHED_NC
    if _CACHED_NC is None:
        _CACHED_NC = _build()
    return _CACHED_NC


def kernel(x, mask, router_w, router_b, W1, b1, W2, b2, _trace=False):
    x = np.asarray(x, np.float32)
    mask = np.asarray(mask, np.float32)
    router_w = np.asarray(router_w, np.float32)
    router_b = np.asarray(router_b, np.float32)
    W1 = np.asarray(W1, np.float32)
    b1 = np.asarray(b1, np.float32)
    W2 = np.asarray(W2, np.float32)
    b2 = np.asarray(b2, np.float32)

    nc = _get_nc()

    # host-side layout prep (sharding): transposes only, no reductions
    xTb_all = np.ascontiguousarray(
        x.transpose(0, 2, 1)).astype(ml_dtypes.bfloat16)       # [B, D, L]
    W1T_all = W1.transpose(0, 2, 1)                            # [E, D, H]
    W2T_all = W2.transpose(0, 2, 1)                            # [E, H, D]
    rwT = np.ascontiguousarray(router_w.T).astype(ml_dtypes.bfloat16)  # [D, E]
    rbc = np.ascontiguousarray(router_b.reshape(E, 1))
    b1T_full = np.ascontiguousarray(b1.T)                      # [H, E]
    b2T = np.ascontiguousarray(b2.T)                           # [D, E]

    pp = np.arange(P)
    gmaskc = (pp[:, None] // 8 == np.arange(16)[None, :]).astype(np.float32)
    emapc = (pp[:, None] % 8 == np.arange(E)[None, :]).astype(np.float32)

    in_maps = []
    for c in range(8):
        g, r = c // 4, c % 4
        hs = slice(r * HS, (r + 1) * HS)
        own = np.zeros((NB, 1), np.float32)
        own[r, 0] = 1.0
        msh = mask[g * NB:(g + 1) * NB]                        # [NB, L]
        # W1b[sb, dc, (g,e), h] = W1T_sh[e, sb*128+dc*16+g, h]
        W1sh = W1T_all[:, :, hs]                               # [E, D, HS]
        W1b = np.ascontiguousarray(
            W1sh.reshape(E, DS, 8, 16, HS).transpose(1, 2, 3, 0, 4).reshape(
                DS, 8, P, HS)).astype(np.float32)
        W2sh = W2T_all[:, hs, :]                               # [E, HS, D]
        W2b = np.ascontiguousarray(
            W2sh.reshape(E, HSUB, 8, 16, D).transpose(1, 2, 3, 0, 4).reshape(
                HSUB, 8, P, D)).astype(np.float32)
        in_maps.append({
            "xTb": xTb_all[g * NB:(g + 1) * NB],
            "maskT": np.ascontiguousarray(msh.T),
            "maskR8": np.ascontiguousarray(
                np.broadcast_to(msh[:, None, :], (NB, E, L))),
            "rwT": rwT,
            "rb": rbc,
            "W1b": W1b,
            "W2b": W2b,
            "b1T": np.ascontiguousarray(b1T_full[hs]),
            "b2T": b2T,
            "ownc": own,
            "gmaskc": gmaskc,
            "emapc": emapc,
        })

    res = run_bass_kernel_spmd(nc, in_maps, core_ids=list(range(8)),
                               trace=_trace)

    out = np.empty((B, L, D), np.float32)
    for g in range(2):
        acc = res.results[g * 4]["outp"].copy()
        for r in range(1, 4):
            acc += res.results[g * 4 + r]["outp"]
        for j in range(NB):
            out[g * NB + j] = acc[j].T
    if _trace:
        return out, res
    return out


# revision 21
# speedup vs baseline: 1.0458x; 1.0458x over previous
"""SMEAR MoE layer (nn_MoELayer_SMEAR) Trainium2 Bass kernel.

Problem: B=8, L=2048, D=1024, H=4096, E=8, fp32.
  logits = x @ router_w.T + router_b; probs = softmax(logits) * mask
  up = probs.sum(L) / clip(mask.sum(L), 1)            # [B, E]
  mW1 = up @ W1 ; mb1 = up @ b1 ; mW2 = up @ W2 ; mb2 = up @ b2  (merged per b)
  out = relu(x @ mW1.T + mb1) @ mW2.T + mb2

Sharding (8 cores): dp=2 over B x tp=4 over H.
  core c: group g=c//4 handles batches g*4..g*4+3; rank r=c%4 handles
  H-shard [r*1024,(r+1)*1024). Each core computes partial outputs for its
  4 batches over its H-shard; host sums the 4 partials per group and
  transposes ([o,t] -> [t,o]) to unshard.

Device phases per core:
  B) router, transpose-free: logits^T [e,t] per 512 chunk (PE bf16), exp
     with +bias (ACT), per-token Z via ones8 matmul (replicated on 8
     partitions), 1/Z * mask * exp reduced over t on DVE -> up[8,b].
  C) merge via block-diagonal matmul: stationary S[128,64] f32r holds
     upT[8,4] in 16 g-blocks (S[(g,e),(b,g)] = up[b,e]); moving = f32r W
     chunks laid out [.., (g,e)=128, free]. One pass of W merges ALL 4
     batches at once (~27us PE per tensor). PSUM [64,512] -> SBUF stage
     (mW1: bf16, mW2: f32r) -> DRAM, read back per batch as [128, sub, n]
     stationary tiles.
  D) MLP per b: hiddenT = relu(mW1^T x^T + b): bf16 stationary x bf16
     moving, hid kept f32r; outT = mW2^T hiddenT + b2 all-f32r (owner core
     adds b2); partials to DRAM. W2 merge is emitted inside b0's shadow.
"""

import numpy as np
import ml_dtypes

import concourse.bass as bass
import concourse.bacc as bacc
import concourse.mybir as mybir
import concourse.tile as tile
from concourse.bass_utils import run_bass_kernel_spmd
from concourse.masks import make_identity

P = 128
B, L, D, H, E = 8, 2048, 1024, 4096, 8
NB = 4          # batches per core
HS = H // 4     # h-shard width per core
DS = D // P     # 8 d-subtiles
HSUB = HS // P  # 8 h-subtiles in shard
OSUB = D // P   # 8 output subtiles
TCH = 512       # moving-dim chunk for matmuls
TC = L // TCH   # 4 chunks per batch

F32 = mybir.dt.float32
F32R = mybir.dt.float32r
BF16 = mybir.dt.bfloat16
AF = mybir.ActivationFunctionType
ALU = mybir.AluOpType
AX = mybir.AxisListType

_CACHED_NC = None


def _build():
    nc = bacc.Bacc("TRN2", target_bir_lowering=False, debug=False)

    xTb = nc.dram_tensor("xTb", [NB, D, L], BF16, kind="ExternalInput")
    maskT = nc.dram_tensor("maskT", [L, NB], F32, kind="ExternalInput")
    maskR8 = nc.dram_tensor("maskR8", [NB, E, L], F32, kind="ExternalInput")
    rwT = nc.dram_tensor("rwT", [D, E], BF16, kind="ExternalInput")
    rb = nc.dram_tensor("rb", [E, 1], F32, kind="ExternalInput")
    # W1b[sb, dc, (g,e), h] = W1T_shard[e, sb*128 + dc*16 + g, h]  (f32r)
    W1b = nc.dram_tensor("W1b", [DS, 8, P, HS], F32R, kind="ExternalInput")
    # W2b[sb, hc, (g,e), o] = W2T_shard[e, sb*128 + hc*16 + g, o]  (f32r)
    W2b = nc.dram_tensor("W2b", [HSUB, 8, P, D], F32R, kind="ExternalInput")
    b1T = nc.dram_tensor("b1T", [HS, E], F32, kind="ExternalInput")
    b2T = nc.dram_tensor("b2T", [D, E], F32, kind="ExternalInput")
    ownc = nc.dram_tensor("ownc", [NB, 1], F32, kind="ExternalInput")
    gmaskc = nc.dram_tensor("gmaskc", [P, 16], F32, kind="ExternalInput")
    emapc = nc.dram_tensor("emapc", [P, E], F32, kind="ExternalInput")
    outp = nc.dram_tensor("outp", [NB, D, L], F32, kind="ExternalOutput")

    # merged weights round trip: [b, sb, c, g, free]
    mW1d = nc.dram_tensor("mW1d", [NB, DS, 8, 16, HS], BF16)
    mW2d = nc.dram_tensor("mW2d", [NB, HSUB, 8, 16, D], F32R)

    with tile.TileContext(nc) as tc:
        with tc.tile_pool(name="const", bufs=1) as const:
            ident = const.tile([P, P], F32)
            make_identity(nc, ident)
            ones_col = const.tile([P, 1], F32)
            nc.gpsimd.memset(ones_col[:], 1.0)
            ones_row = const.tile([1, P], F32)
            nc.gpsimd.memset(ones_row[:], 1.0)
            ones8f = const.tile([E, E], F32)
            nc.gpsimd.memset(ones8f[:], 1.0)
            ones8 = const.tile([E, E], F32R)
            nc.vector.tensor_copy(ones8[:], ones8f[:])

            rwT_sb = const.tile([P, DS, E], BF16)
            nc.sync.dma_start(rwT_sb[:], rwT.ap().rearrange("(s p) e -> p s e", p=P))
            rb_sb = const.tile([E, 1], F32)
            nc.sync.dma_start(rb_sb[:], rb.ap())
            maskT_sb = const.tile([P, L // P, NB], F32)
            nc.sync.dma_start(maskT_sb[:], maskT.ap().rearrange("(q p) b -> p q b", p=P))
            b1T_sb = const.tile([P, HSUB, E], F32)
            nc.sync.dma_start(b1T_sb[:], b1T.ap().rearrange("(s p) e -> p s e", p=P))
            b2T_sb = const.tile([P, OSUB, E], F32)
            nc.sync.dma_start(b2T_sb[:], b2T.ap().rearrange("(s p) e -> p s e", p=P))
            own_sb = const.tile([NB, 1], F32)
            nc.sync.dma_start(own_sb[:], ownc.ap())
            gmask_sb = const.tile([P, 16], F32)
            nc.sync.dma_start(gmask_sb[:], gmaskc.ap())
            emap_sb = const.tile([P, E], F32)
            nc.sync.dma_start(emap_sb[:], emapc.ap())

            up_sb = const.tile([E, NB], F32)
            upT_sb = const.tile([NB, E], F32)
            upTo_sb = const.tile([NB, E], F32)
            up_bc = const.tile([P, NB, E], F32)
            upo_bc = const.tile([P, NB, E], F32)
            mb1_sb = const.tile([P, NB, HSUB], F32)
            mb2_sb = const.tile([P, NB, OSUB], F32)
            invbc_sb = const.tile([P, NB], F32)
            S2_sb = const.tile([P, NB * 16], F32R)
            upRep = const.tile([P, NB], F32)
            tmp8 = const.tile([P, E], F32)

            # ---------------- Phase B: router ----------------
            with tc.tile_pool(name="rpsum", bufs=1, space="PSUM") as rpsum, \
                 tc.tile_pool(name="rsb", bufs=6) as rsb, \
                 tc.tile_pool(name="xrt", bufs=4) as xrt, \
                 tc.tile_pool(name="lgp", bufs=3, space="PSUM") as lgp, \
                 tc.tile_pool(name="dnp", bufs=2, space="PSUM") as dnp:

                # denominators: denom[b] = clip(sum_t mask, 1); invbc = 1/denom bcast
                mpart = rsb.tile([P, NB], F32)
                for b in range(NB):
                    nc.vector.tensor_reduce(
                        mpart[:, b:b + 1], maskT_sb[:, :, b], axis=AX.X, op=ALU.add)
                den_ps = rpsum.tile([NB, 1], F32, tag="rps")
                nc.tensor.matmul(den_ps[:], mpart[:], ones_col[:], start=True, stop=True)
                den_sb = rsb.tile([NB, 1], F32)
                nc.vector.tensor_scalar_max(den_sb[:], den_ps[:], 1.0)
                inv_sb = rsb.tile([NB, 1], F32)
                nc.vector.reciprocal(inv_sb[:], den_sb[:])
                invT_ps = rpsum.tile([1, NB], F32, tag="rps")
                nc.tensor.transpose(invT_ps[:], inv_sb[:], ident[:NB, :NB])
                invT_sb = rsb.tile([1, NB], F32)
                nc.vector.tensor_copy(invT_sb[:], invT_ps[:])
                invbc_ps = rpsum.tile([P, NB], F32, tag="rps")
                nc.tensor.matmul(invbc_ps[:], ones_row[:], invT_sb[:], start=True, stop=True)
                nc.vector.tensor_copy(invbc_sb[:], invbc_ps[:])

                # up[b] = invb * sum_t exp[e,t] * mask[t] / Z[t], computed in
                # [e,t] orientation (no transposes). The Z matmul for chunk j
                # is emitted after the logits matmuls of chunk j+1 so the PE
                # never waits on ACT exp.
                def den_stage(j, exs, m8, b):
                    dn_ps = dnp.tile([E, TCH], F32, tag="dn")
                    nc.tensor.matmul(dn_ps[:], ones8[:], exs[j][:],
                                     start=True, stop=True)
                    zm = rsb.tile([E, TCH], F32, tag="zm")
                    nc.vector.reciprocal(zm[:], dn_ps[:])
                    nc.vector.tensor_tensor(
                        zm[:], zm[:], m8[:, j * TCH:(j + 1) * TCH], ALU.mult)
                    t1 = rsb.tile([E, TCH], F32, tag="t1")
                    nc.vector.tensor_tensor(t1[:], exs[j][:], zm[:], ALU.mult)
                    red = rsb.tile([E, 1], F32, tag="red")
                    nc.vector.tensor_reduce(red[:], t1[:], axis=AX.X, op=ALU.add)
                    if j == 0:
                        nc.vector.tensor_copy(up_sb[:, b:b + 1], red[:])
                    else:
                        nc.vector.tensor_tensor(
                            up_sb[:, b:b + 1], up_sb[:, b:b + 1], red[:], ALU.add)

                for b in range(NB):
                    m8 = rsb.tile([E, L], F32, tag="m8")
                    nc.sync.dma_start(m8[:], maskR8.ap()[b])
                    exs = [None] * TC
                    for t4 in range(TC):
                        xt = xrt.tile([P, DS, TCH], BF16, tag="xrt")
                        nc.sync.dma_start(
                            xt[:],
                            xTb.ap()[b].rearrange("(s p) t -> p s t", p=P)[
                                :, :, t4 * TCH:(t4 + 1) * TCH])
                        lg_ps = lgp.tile([E, TCH], F32, tag="lg")
                        for dsb in range(DS):
                            nc.tensor.matmul(lg_ps[:], rwT_sb[:, dsb], xt[:, dsb],
                                             start=(dsb == 0), stop=(dsb == DS - 1))
                        ex = rsb.tile([E, TCH], F32R, tag="ex")
                        nc.scalar.activation(ex[:], lg_ps[:], AF.Exp, bias=rb_sb[:])
                        exs[t4] = ex
                        if t4 >= 1:
                            den_stage(t4 - 1, exs, m8, b)
                    den_stage(TC - 1, exs, m8, b)
                    nc.vector.tensor_scalar_mul(
                        up_sb[:, b:b + 1], up_sb[:, b:b + 1], invbc_sb[:E, b:b + 1])

                # broadcast up across partitions; owner-masked copy for b2
                upT_ps = rpsum.tile([NB, E], F32, tag="rps")
                nc.tensor.transpose(upT_ps[:], up_sb[:], ident[:E, :E])
                nc.vector.tensor_copy(upT_sb[:], upT_ps[:])
                nc.vector.tensor_scalar_mul(upTo_sb[:], upT_sb[:], own_sb[:])
                for b in range(NB):
                    rowu = rsb.tile([1, E], F32, tag="rowu")
                    nc.sync.dma_start(rowu[:], upT_sb[b:b + 1, :])
                    rowo = rsb.tile([1, E], F32, tag="rowo")
                    nc.sync.dma_start(rowo[:], upTo_sb[b:b + 1, :])
                    bc_ps = rpsum.tile([P, E], F32, tag="rps")
                    nc.tensor.matmul(bc_ps[:], ones_row[:], rowu[:], start=True, stop=True)
                    nc.vector.tensor_copy(up_bc[:, b], bc_ps[:])
                    bo_ps = rpsum.tile([P, E], F32, tag="rps")
                    nc.tensor.matmul(bo_ps[:], ones_row[:], rowo[:], start=True, stop=True)
                    nc.vector.tensor_copy(upo_bc[:, b], bo_ps[:])

                # merged biases: mb1[b] = sum_e up[b,e] b1T[:,e]; mb2 owner-masked
                for b in range(NB):
                    nc.vector.tensor_scalar_mul(
                        mb1_sb[:, b], b1T_sb[:, :, 0], up_bc[:, b, 0:1])
                    nc.vector.tensor_scalar_mul(
                        mb2_sb[:, b], b2T_sb[:, :, 0], upo_bc[:, b, 0:1])
                    for e in range(1, E):
                        nc.vector.scalar_tensor_tensor(
                            mb1_sb[:, b], b1T_sb[:, :, e], up_bc[:, b, e:e + 1],
                            mb1_sb[:, b], ALU.mult, ALU.add)
                        nc.vector.scalar_tensor_tensor(
                            mb2_sb[:, b], b2T_sb[:, :, e], upo_bc[:, b, e:e + 1],
                            mb2_sb[:, b], ALU.mult, ALU.add)

                # block-diagonal merge stationary S[(g,e), b*16+g] = up[b,e]
                for b in range(NB):
                    nc.vector.tensor_tensor(
                        tmp8[:], up_bc[:, b], emap_sb[:], ALU.mult)
                    nc.vector.tensor_reduce(
                        upRep[:, b:b + 1], tmp8[:], axis=AX.X, op=ALU.add)
                    nc.vector.tensor_scalar_mul(
                        S2_sb[:, b * 16:(b + 1) * 16], gmask_sb[:], upRep[:, b:b + 1])

            # ---------------- Phases C+D: merge + MLP ----------------
            with tc.tile_pool(name="mrgps", bufs=4, space="PSUM") as mrgps, \
                 tc.tile_pool(name="mwp", bufs=1) as mwp, \
                 tc.tile_pool(name="hidp", bufs=1) as hidp, \
                 tc.tile_pool(name="xtp", bufs=4) as xtp, \
                 tc.tile_pool(name="osbp", bufs=2) as osbp, \
                 tc.tile_pool(name="mmp", bufs=4, space="PSUM") as mmp:

                def emit_merge(Wb, mWd, dt_stage, nch, spc, wch, stgp,
                               dve_only=False):
                    """One f32r pass of W merges all 4 batches; a stage tile
                    spans spc chunk iterations, then one scatter per batch."""
                    for sb in range(8):
                        stg = None
                        for cq in range(8 // nch):
                            ch = wch.tile([P, nch, 1024], F32R, tag="wch")
                            nc.sync.dma_start(
                                ch[:],
                                Wb.ap()[sb, cq * nch:(cq + 1) * nch].rearrange(
                                    "c p h -> p c h"))
                            if cq % spc == 0:
                                stg = stgp.tile([64, spc * nch, 1024], dt_stage,
                                                tag="stg")
                            for c in range(nch):
                                cs = (cq % spc) * nch + c
                                for hh in range(2):
                                    ps = mrgps.tile([64, TCH], F32, tag="mps")
                                    nc.tensor.matmul(
                                        ps[:], S2_sb[:],
                                        ch[:, c, hh * TCH:(hh + 1) * TCH],
                                        start=True, stop=True)
                                    if dve_only or (cs + hh) % 2 == 0:
                                        nc.vector.tensor_copy(
                                            stg[:, cs, hh * TCH:(hh + 1) * TCH],
                                            ps[:])
                                    else:
                                        nc.scalar.activation(
                                            stg[:, cs, hh * TCH:(hh + 1) * TCH],
                                            ps[:], AF.Identity)
                            if cq % spc == spc - 1:
                                c0 = (cq - spc + 1) * nch
                                for b in range(NB):
                                    nc.sync.dma_start(
                                        mWd.ap()[b, sb, c0:c0 + spc * nch]
                                        .rearrange("c g h -> g c h"),
                                        stg[b * 16:(b + 1) * 16])

                with tc.tile_pool(name="w1ch", bufs=3) as w1ch, \
                     tc.tile_pool(name="stg1", bufs=2) as stg1p:
                    emit_merge(W1b, mW1d, BF16, 2, 2, w1ch, stg1p)
                # prefetch b0 merged weights with a long separation from
                # their consumers (the readback DMA completion is not a
                # reliable gate for immediately-following matmuls)
                mw1 = mwp.tile([P, DS, HS], BF16, tag="mw1")
                nc.sync.dma_start(
                    mw1[:], mW1d.ap()[0].rearrange("s c g h -> c g s h"))
                with tc.tile_pool(name="w2ch", bufs=2) as w2ch, \
                     tc.tile_pool(name="stg2", bufs=2) as stg2p:
                    emit_merge(W2b, mW2d, F32R, 2, 2, w2ch, stg2p)
                mw2 = mwp.tile([P, HSUB, D], F32R, tag="mw2")
                nc.sync.dma_start(
                    mw2[:], mW2d.ap()[0].rearrange("s c g h -> c g s h"))

                for b in range(NB):
                    hid = hidp.tile([P, HSUB, L], F32R, tag="hid")
                    # b0's first two chunks are recomputed at the end: their
                    # relu psum drains collide with the merge-copy backlog on
                    # the drain engines (bank reuse outruns the delayed reads)
                    t4_order = [0, 1, 2, 3, 0, 1] if b == 0 else range(TC)
                    for t4 in t4_order:
                        xt = xtp.tile([P, DS, TCH], BF16, tag="xt")
                        nc.sync.dma_start(
                            xt[:],
                            xTb.ap()[b].rearrange("(s p) t -> p s t", p=P)[
                                :, :, t4 * TCH:(t4 + 1) * TCH])
                        for hb in range(HSUB):
                            ps = mmp.tile([P, TCH], F32, tag="ps")
                            for dsb in range(DS):
                                nc.tensor.matmul(
                                    ps[:], mw1[:, dsb, hb * P:(hb + 1) * P],
                                    xt[:, dsb],
                                    start=(dsb == 0), stop=(dsb == DS - 1))
                            nc.scalar.activation(
                                hid[:, hb, t4 * TCH:(t4 + 1) * TCH], ps[:],
                                AF.Relu, bias=mb1_sb[:, b, hb:hb + 1])
                    if b + 1 < NB:
                        mw1n = mwp.tile([P, DS, HS], BF16, tag="mw1")
                        nc.sync.dma_start(
                            mw1n[:],
                            mW1d.ap()[b + 1].rearrange("s c g h -> c g s h"))
                    for t4 in range(TC):
                        for obq in range(4):
                            ot = osbp.tile([P, 2, TCH], F32, tag="ot")
                            for oo in range(2):
                                ob = obq * 2 + oo
                                ps = mmp.tile([P, TCH], F32, tag="ps")
                                for hs in range(HSUB):
                                    nc.tensor.matmul(
                                        ps[:], mw2[:, hs, ob * P:(ob + 1) * P],
                                        hid[:, hs, t4 * TCH:(t4 + 1) * TCH],
                                        start=(hs == 0), stop=(hs == HSUB - 1))
                                nc.vector.tensor_scalar_add(
                                    ot[:, oo], ps[:], mb2_sb[:, b, ob:ob + 1])
                            nc.sync.dma_start(
                                outp.ap()[b, obq * 256:(obq + 1) * 256,
                                          t4 * TCH:(t4 + 1) * TCH].rearrange(
                                              "(o p) t -> p o t", p=P),
                                ot[:])
                    if b + 1 < NB:
                        mw2n = mwp.tile([P, HSUB, D], F32R, tag="mw2")
                        nc.sync.dma_start(
                            mw2n[:],
                            mW2d.ap()[b + 1].rearrange("s c g h -> c g s h"))
                        mw1, mw2 = mw1n, mw2n

    nc.compile()
    return nc


def _get_nc():
    global _CACHED_NC
    if _CACHED_NC is None:
        _CACHED_NC = _build()
    return _CACHED_NC


def kernel(x, mask, router_w, router_b, W1, b1, W2, b2, _trace=False):
    x = np.asarray(x, np.float32)
    mask = np.asarray(mask, np.float32)
    router_w = np.asarray(router_w, np.float32)
    router_b = np.asarray(router_b, np.float32)
    W1 = np.asarray(W1, np.float32)
    b1 = np.asarray(b1, np.float32)
    W2 = np.asarray(W2, np.float32)
    b2 = np.asarray(b2, np.float32)

    nc = _get_nc()

    # host-side layout prep (sharding): transposes only, no reductions
    xTb_all = np.ascontiguousarray(
        x.transpose(0, 2, 1)).astype(ml_dtypes.bfloat16)       # [B, D, L]
    W1T_all = W1.transpose(0, 2, 1)                            # [E, D, H]
    W2T_all = W2.transpose(0, 2, 1)                            # [E, H, D]
    rwT = np.ascontiguousarray(router_w.T).astype(ml_dtypes.bfloat16)  # [D, E]
    rbc = np.ascontiguousarray(router_b.reshape(E, 1))
    b1T_full = np.ascontiguousarray(b1.T)                      # [H, E]
    b2T = np.ascontiguousarray(b2.T)                           # [D, E]

    pp = np.arange(P)
    gmaskc = (pp[:, None] // 8 == np.arange(16)[None, :]).astype(np.float32)
    emapc = (pp[:, None] % 8 == np.arange(E)[None, :]).astype(np.float32)

    in_maps = []
    for c in range(8):
        g, r = c // 4, c % 4
        hs = slice(r * HS, (r + 1) * HS)
        own = np.zeros((NB, 1), np.float32)
        own[r, 0] = 1.0
        msh = mask[g * NB:(g + 1) * NB]                        # [NB, L]
        # W1b[sb, dc, (g,e), h] = W1T_sh[e, sb*128+dc*16+g, h]
        W1sh = W1T_all[:, :, hs]                               # [E, D, HS]
        W1b = np.ascontiguousarray(
            W1sh.reshape(E, DS, 8, 16, HS).transpose(1, 2, 3, 0, 4).reshape(
                DS, 8, P, HS)).astype(np.float32)
        W2sh = W2T_all[:, hs, :]                               # [E, HS, D]
        W2b = np.ascontiguousarray(
            W2sh.reshape(E, HSUB, 8, 16, D).transpose(1, 2, 3, 0, 4).reshape(
                HSUB, 8, P, D)).astype(np.float32)
        in_maps.append({
            "xTb": xTb_all[g * NB:(g + 1) * NB],
            "maskT": np.ascontiguousarray(msh.T),
            "maskR8": np.ascontiguousarray(
                np.broadcast_to(msh[:, None, :], (NB, E, L))),
            "rwT": rwT,
            "rb": rbc,
            "W1b": W1b,
            "W2b": W2b,
            "b1T": np.ascontiguousarray(b1T_full[hs]),
            "b2T": b2T,
            "ownc": own,
            "gmaskc": gmaskc,
            "emapc": emapc,
        })

    res = run_bass_kernel_spmd(nc, in_maps, core_ids=list(range(8)),
                               trace=_trace)

    out = np.empty((B, L, D), np.float32)
    for g in range(2):
        acc = res.results[g * 4]["outp"].copy()
        for r in range(1, 4):
            acc += res.results[g * 4 + r]["outp"]
        for j in range(NB):
            out[g * NB + j] = acc[j].T
    if _trace:
        return out, res
    return out


# revision 22
# speedup vs baseline: 1.1220x; 1.0728x over previous
"""SMEAR MoE layer (nn_MoELayer_SMEAR) Trainium2 Bass kernel.

Problem: B=8, L=2048, D=1024, H=4096, E=8, fp32.
  logits = x @ router_w.T + router_b; probs = softmax(logits) * mask
  up = probs.sum(L) / clip(mask.sum(L), 1)            # [B, E]
  mW1 = up @ W1 ; mb1 = up @ b1 ; mW2 = up @ W2 ; mb2 = up @ b2  (merged per b)
  out = relu(x @ mW1.T + mb1) @ mW2.T + mb2

Sharding (8 cores): dp=2 over B x tp=4 over H.
  core c: group g=c//4 handles batches g*4..g*4+3; rank r=c%4 handles
  H-shard [r*1024,(r+1)*1024). Each core computes partial outputs for its
  4 batches over its H-shard; host sums the 4 partials per group and
  transposes ([o,t] -> [t,o]) to unshard.

Device phases per core:
  B) router, transpose-free: logits^T [e,t] per 512 chunk (PE bf16), exp
     with +bias (ACT), per-token Z via ones8 matmul (replicated on 8
     partitions), 1/Z * mask * exp reduced over t on DVE -> up[8,b].
  C) merge via block-diagonal matmul: stationary S[128,64] f32r holds
     upT[8,4] in 16 g-blocks (S[(g,e),(b,g)] = up[b,e]); moving = f32r W
     chunks laid out [.., (g,e)=128, free]. One pass of W merges ALL 4
     batches at once (~27us PE per tensor). PSUM [64,512] -> SBUF stage
     (mW1: bf16, mW2: f32r) -> DRAM, read back per batch as [128, sub, n]
     stationary tiles.
  D) MLP per b: hiddenT = relu(mW1^T x^T + b): bf16 stationary x bf16
     moving, hid kept f32r; outT = mW2^T hiddenT + b2 all-f32r (owner core
     adds b2); partials to DRAM. W2 merge is emitted inside b0's shadow.
"""

import numpy as np
import ml_dtypes

import concourse.bass as bass
import concourse.bacc as bacc
import concourse.mybir as mybir
import concourse.tile as tile
from concourse.bass_utils import run_bass_kernel_spmd
from concourse.masks import make_identity

P = 128
B, L, D, H, E = 8, 2048, 1024, 4096, 8
NB = 4          # batches per core
HS = H // 4     # h-shard width per core
DS = D // P     # 8 d-subtiles
HSUB = HS // P  # 8 h-subtiles in shard
OSUB = D // P   # 8 output subtiles
TCH = 512       # moving-dim chunk for matmuls
TC = L // TCH   # 4 chunks per batch

F32 = mybir.dt.float32
F32R = mybir.dt.float32r
BF16 = mybir.dt.bfloat16
AF = mybir.ActivationFunctionType
ALU = mybir.AluOpType
AX = mybir.AxisListType

_CACHED_NC = None


def _build():
    nc = bacc.Bacc("TRN2", target_bir_lowering=False, debug=False)

    xTb = nc.dram_tensor("xTb", [NB, D, L], BF16, kind="ExternalInput")
    maskT = nc.dram_tensor("maskT", [L, NB], F32, kind="ExternalInput")
    maskR8 = nc.dram_tensor("maskR8", [NB, E, L], F32, kind="ExternalInput")
    rwT = nc.dram_tensor("rwT", [D, E], BF16, kind="ExternalInput")
    rb = nc.dram_tensor("rb", [E, 1], F32, kind="ExternalInput")
    # W1b[sb, dc, (g,e), h] = W1T_shard[e, sb*128 + dc*16 + g, h]  (f32r)
    W1b = nc.dram_tensor("W1b", [DS, 8, P, HS], F32R, kind="ExternalInput")
    # W2b[sb, hc, (g,e), o] = W2T_shard[e, sb*128 + hc*16 + g, o]  (f32r)
    W2b = nc.dram_tensor("W2b", [HSUB, 8, P, D], F32R, kind="ExternalInput")
    b1T = nc.dram_tensor("b1T", [HS, E], F32, kind="ExternalInput")
    b2T = nc.dram_tensor("b2T", [D, E], F32, kind="ExternalInput")
    ownc = nc.dram_tensor("ownc", [NB, 1], F32, kind="ExternalInput")
    gmaskc = nc.dram_tensor("gmaskc", [P, 16], F32, kind="ExternalInput")
    emapc = nc.dram_tensor("emapc", [P, E], F32, kind="ExternalInput")
    outp = nc.dram_tensor("outp", [NB, D, L], F32, kind="ExternalOutput")

    # merged weights round trip: [b, sb, c, g, free]
    mW1d = nc.dram_tensor("mW1d", [NB, DS, 8, 16, HS], BF16)
    mW2d = nc.dram_tensor("mW2d", [NB, HSUB, 8, 16, D], F32R)

    with tile.TileContext(nc) as tc:
        with tc.tile_pool(name="const", bufs=1) as const:
            ident = const.tile([P, P], F32)
            make_identity(nc, ident)
            ones_col = const.tile([P, 1], F32)
            nc.gpsimd.memset(ones_col[:], 1.0)
            ones_row = const.tile([1, P], F32)
            nc.gpsimd.memset(ones_row[:], 1.0)
            ones8f = const.tile([E, E], F32)
            nc.gpsimd.memset(ones8f[:], 1.0)
            ones8 = const.tile([E, E], F32R)
            nc.vector.tensor_copy(ones8[:], ones8f[:])

            rwT_sb = const.tile([P, DS, E], BF16)
            nc.sync.dma_start(rwT_sb[:], rwT.ap().rearrange("(s p) e -> p s e", p=P))
            rb_sb = const.tile([E, 1], F32)
            nc.sync.dma_start(rb_sb[:], rb.ap())
            maskT_sb = const.tile([P, L // P, NB], F32)
            nc.sync.dma_start(maskT_sb[:], maskT.ap().rearrange("(q p) b -> p q b", p=P))
            b1T_sb = const.tile([P, HSUB, E], F32)
            nc.sync.dma_start(b1T_sb[:], b1T.ap().rearrange("(s p) e -> p s e", p=P))
            b2T_sb = const.tile([P, OSUB, E], F32)
            nc.sync.dma_start(b2T_sb[:], b2T.ap().rearrange("(s p) e -> p s e", p=P))
            own_sb = const.tile([NB, 1], F32)
            nc.sync.dma_start(own_sb[:], ownc.ap())
            gmask_sb = const.tile([P, 16], F32)
            nc.sync.dma_start(gmask_sb[:], gmaskc.ap())
            emap_sb = const.tile([P, E], F32)
            nc.sync.dma_start(emap_sb[:], emapc.ap())

            up_sb = const.tile([E, NB], F32)
            upT_sb = const.tile([NB, E], F32)
            upTo_sb = const.tile([NB, E], F32)
            up_bc = const.tile([P, NB, E], F32)
            upo_bc = const.tile([P, NB, E], F32)
            mb1_sb = const.tile([P, NB, HSUB], F32)
            mb2_sb = const.tile([P, NB, OSUB], F32)
            invbc_sb = const.tile([P, NB], F32)
            S2_sb = const.tile([P, NB * 16], F32R)
            upRep = const.tile([P, NB], F32)
            tmp8 = const.tile([P, E], F32)

            # ---------------- Phase B: router ----------------
            with tc.tile_pool(name="rpsum", bufs=1, space="PSUM") as rpsum, \
                 tc.tile_pool(name="rsb", bufs=6) as rsb, \
                 tc.tile_pool(name="xrt", bufs=4) as xrt, \
                 tc.tile_pool(name="lgp", bufs=3, space="PSUM") as lgp, \
                 tc.tile_pool(name="dnp", bufs=2, space="PSUM") as dnp:

                # denominators: denom[b] = clip(sum_t mask, 1); invbc = 1/denom bcast
                mpart = rsb.tile([P, NB], F32)
                for b in range(NB):
                    nc.vector.tensor_reduce(
                        mpart[:, b:b + 1], maskT_sb[:, :, b], axis=AX.X, op=ALU.add)
                den_ps = rpsum.tile([NB, 1], F32, tag="rps")
                nc.tensor.matmul(den_ps[:], mpart[:], ones_col[:], start=True, stop=True)
                den_sb = rsb.tile([NB, 1], F32)
                nc.vector.tensor_scalar_max(den_sb[:], den_ps[:], 1.0)
                inv_sb = rsb.tile([NB, 1], F32)
                nc.vector.reciprocal(inv_sb[:], den_sb[:])
                invT_ps = rpsum.tile([1, NB], F32, tag="rps")
                nc.tensor.transpose(invT_ps[:], inv_sb[:], ident[:NB, :NB])
                invT_sb = rsb.tile([1, NB], F32)
                nc.vector.tensor_copy(invT_sb[:], invT_ps[:])
                invbc_ps = rpsum.tile([P, NB], F32, tag="rps")
                nc.tensor.matmul(invbc_ps[:], ones_row[:], invT_sb[:], start=True, stop=True)
                nc.vector.tensor_copy(invbc_sb[:], invbc_ps[:])

                # up[b] = invb * sum_t exp[e,t] * mask[t] / Z[t], computed in
                # [e,t] orientation (no transposes). The Z matmul for chunk j
                # is emitted after the logits matmuls of chunk j+1 so the PE
                # never waits on ACT exp.
                def den_stage(j, exs, m8, b):
                    dn_ps = dnp.tile([E, TCH], F32, tag="dn")
                    nc.tensor.matmul(dn_ps[:], ones8[:], exs[j][:],
                                     start=True, stop=True)
                    zm = rsb.tile([E, TCH], F32, tag="zm")
                    nc.vector.reciprocal(zm[:], dn_ps[:])
                    nc.vector.tensor_tensor(
                        zm[:], zm[:], m8[:, j * TCH:(j + 1) * TCH], ALU.mult)
                    t1 = rsb.tile([E, TCH], F32, tag="t1")
                    nc.vector.tensor_tensor(t1[:], exs[j][:], zm[:], ALU.mult)
                    red = rsb.tile([E, 1], F32, tag="red")
                    nc.vector.tensor_reduce(red[:], t1[:], axis=AX.X, op=ALU.add)
                    if j == 0:
                        nc.vector.tensor_copy(up_sb[:, b:b + 1], red[:])
                    else:
                        nc.vector.tensor_tensor(
                            up_sb[:, b:b + 1], up_sb[:, b:b + 1], red[:], ALU.add)

                for b in range(NB):
                    m8 = rsb.tile([E, L], F32, tag="m8")
                    nc.sync.dma_start(m8[:], maskR8.ap()[b])
                    exs = [None] * TC
                    for t4 in range(TC):
                        xt = xrt.tile([P, DS, TCH], BF16, tag="xrt")
                        nc.sync.dma_start(
                            xt[:],
                            xTb.ap()[b].rearrange("(s p) t -> p s t", p=P)[
                                :, :, t4 * TCH:(t4 + 1) * TCH])
                        lg_ps = lgp.tile([E, TCH], F32, tag="lg")
                        for dsb in range(DS):
                            nc.tensor.matmul(lg_ps[:], rwT_sb[:, dsb], xt[:, dsb],
                                             start=(dsb == 0), stop=(dsb == DS - 1))
                        ex = rsb.tile([E, TCH], F32R, tag="ex")
                        nc.scalar.activation(ex[:], lg_ps[:], AF.Exp, bias=rb_sb[:])
                        exs[t4] = ex
                        if t4 >= 1:
                            den_stage(t4 - 1, exs, m8, b)
                    den_stage(TC - 1, exs, m8, b)
                    nc.vector.tensor_scalar_mul(
                        up_sb[:, b:b + 1], up_sb[:, b:b + 1], invbc_sb[:E, b:b + 1])

                # broadcast up across partitions; owner-masked copy for b2
                upT_ps = rpsum.tile([NB, E], F32, tag="rps")
                nc.tensor.transpose(upT_ps[:], up_sb[:], ident[:E, :E])
                nc.vector.tensor_copy(upT_sb[:], upT_ps[:])
                nc.vector.tensor_scalar_mul(upTo_sb[:], upT_sb[:], own_sb[:])
                for b in range(NB):
                    rowu = rsb.tile([1, E], F32, tag="rowu")
                    nc.sync.dma_start(rowu[:], upT_sb[b:b + 1, :])
                    rowo = rsb.tile([1, E], F32, tag="rowo")
                    nc.sync.dma_start(rowo[:], upTo_sb[b:b + 1, :])
                    bc_ps = rpsum.tile([P, E], F32, tag="rps")
                    nc.tensor.matmul(bc_ps[:], ones_row[:], rowu[:], start=True, stop=True)
                    nc.vector.tensor_copy(up_bc[:, b], bc_ps[:])
                    bo_ps = rpsum.tile([P, E], F32, tag="rps")
                    nc.tensor.matmul(bo_ps[:], ones_row[:], rowo[:], start=True, stop=True)
                    nc.vector.tensor_copy(upo_bc[:, b], bo_ps[:])

                # merged biases: mb1[b] = sum_e up[b,e] b1T[:,e]; mb2 owner-masked
                for b in range(NB):
                    nc.vector.tensor_scalar_mul(
                        mb1_sb[:, b], b1T_sb[:, :, 0], up_bc[:, b, 0:1])
                    nc.vector.tensor_scalar_mul(
                        mb2_sb[:, b], b2T_sb[:, :, 0], upo_bc[:, b, 0:1])
                    for e in range(1, E):
                        nc.vector.scalar_tensor_tensor(
                            mb1_sb[:, b], b1T_sb[:, :, e], up_bc[:, b, e:e + 1],
                            mb1_sb[:, b], ALU.mult, ALU.add)
                        nc.vector.scalar_tensor_tensor(
                            mb2_sb[:, b], b2T_sb[:, :, e], upo_bc[:, b, e:e + 1],
                            mb2_sb[:, b], ALU.mult, ALU.add)

                # block-diagonal merge stationary S[(g,e), b*16+g] = up[b,e]
                for b in range(NB):
                    nc.vector.tensor_tensor(
                        tmp8[:], up_bc[:, b], emap_sb[:], ALU.mult)
                    nc.vector.tensor_reduce(
                        upRep[:, b:b + 1], tmp8[:], axis=AX.X, op=ALU.add)
                    nc.vector.tensor_scalar_mul(
                        S2_sb[:, b * 16:(b + 1) * 16], gmask_sb[:], upRep[:, b:b + 1])

            # ---------------- Phases C+D: merge + MLP ----------------
            with tc.tile_pool(name="mrgps", bufs=4, space="PSUM") as mrgps, \
                 tc.tile_pool(name="mwp", bufs=1) as mwp, \
                 tc.tile_pool(name="hidp", bufs=1) as hidp, \
                 tc.tile_pool(name="xtp", bufs=4) as xtp, \
                 tc.tile_pool(name="osbp", bufs=2) as osbp, \
                 tc.tile_pool(name="mmp", bufs=4, space="PSUM") as mmp:

                def emit_merge(Wb, mWd, dt_stage, nch, spc, wch, stgp,
                               dve_only=False):
                    """One f32r pass of W merges all 4 batches; a stage tile
                    spans spc chunk iterations, then one scatter per batch."""
                    for sb in range(8):
                        stg = None
                        for cq in range(8 // nch):
                            ch = wch.tile([P, nch, 1024], F32R, tag="wch")
                            nc.sync.dma_start(
                                ch[:],
                                Wb.ap()[sb, cq * nch:(cq + 1) * nch].rearrange(
                                    "c p h -> p c h"))
                            if cq % spc == 0:
                                stg = stgp.tile([64, spc * nch, 1024], dt_stage,
                                                tag="stg")
                            for c in range(nch):
                                cs = (cq % spc) * nch + c
                                for hh in range(2):
                                    ps = mrgps.tile([64, TCH], F32, tag="mps")
                                    nc.tensor.matmul(
                                        ps[:], S2_sb[:],
                                        ch[:, c, hh * TCH:(hh + 1) * TCH],
                                        start=True, stop=True)
                                    if dve_only or (cs + hh) % 2 == 0:
                                        nc.vector.tensor_copy(
                                            stg[:, cs, hh * TCH:(hh + 1) * TCH],
                                            ps[:])
                                    else:
                                        nc.scalar.activation(
                                            stg[:, cs, hh * TCH:(hh + 1) * TCH],
                                            ps[:], AF.Identity)
                            if cq % spc == spc - 1:
                                c0 = (cq - spc + 1) * nch
                                for b in range(NB):
                                    nc.gpsimd.dma_start(
                                        mWd.ap()[b, sb, c0:c0 + spc * nch]
                                        .rearrange("c g h -> g c h"),
                                        stg[b * 16:(b + 1) * 16])

                with tc.tile_pool(name="w1ch", bufs=3) as w1ch, \
                     tc.tile_pool(name="stg1", bufs=2) as stg1p:
                    emit_merge(W1b, mW1d, BF16, 2, 2, w1ch, stg1p)
                # prefetch b0 merged weights with a long separation from
                # their consumers (the readback DMA completion is not a
                # reliable gate for immediately-following matmuls)
                mw1 = mwp.tile([P, DS, HS], BF16, tag="mw1")
                nc.sync.dma_start(
                    mw1[:], mW1d.ap()[0].rearrange("s c g h -> c g s h"))
                with tc.tile_pool(name="w2ch", bufs=2) as w2ch, \
                     tc.tile_pool(name="stg2", bufs=2) as stg2p:
                    emit_merge(W2b, mW2d, F32R, 2, 2, w2ch, stg2p)
                mw2 = mwp.tile([P, HSUB, D], F32R, tag="mw2")
                nc.sync.dma_start(
                    mw2[:], mW2d.ap()[0].rearrange("s c g h -> c g s h"))

                for b in range(NB):
                    hid = hidp.tile([P, HSUB, L], F32R, tag="hid")
                    # b0's first two chunks are recomputed at the end: their
                    # relu psum drains collide with the merge-copy backlog on
                    # the drain engines (bank reuse outruns the delayed reads)
                    t4_order = [0, 1, 2, 3, 0, 1] if b == 0 else range(TC)
                    for t4 in t4_order:
                        xt = xtp.tile([P, DS, TCH], BF16, tag="xt")
                        nc.sync.dma_start(
                            xt[:],
                            xTb.ap()[b].rearrange("(s p) t -> p s t", p=P)[
                                :, :, t4 * TCH:(t4 + 1) * TCH])
                        for hb in range(HSUB):
                            ps = mmp.tile([P, TCH], F32, tag="ps")
                            for dsb in range(DS):
                                nc.tensor.matmul(
                                    ps[:], mw1[:, dsb, hb * P:(hb + 1) * P],
                                    xt[:, dsb],
                                    start=(dsb == 0), stop=(dsb == DS - 1))
                            nc.scalar.activation(
                                hid[:, hb, t4 * TCH:(t4 + 1) * TCH], ps[:],
                                AF.Relu, bias=mb1_sb[:, b, hb:hb + 1])
                    if b + 1 < NB:
                        mw1n = mwp.tile([P, DS, HS], BF16, tag="mw1")
                        nc.sync.dma_start(
                            mw1n[:],
                            mW1d.ap()[b + 1].rearrange("s c g h -> c g s h"))
                    for t4 in range(TC):
                        for obq in range(4):
                            ot = osbp.tile([P, 2, TCH], F32, tag="ot")
                            for oo in range(2):
                                ob = obq * 2 + oo
                                ps = mmp.tile([P, TCH], F32, tag="ps")
                                for hs in range(HSUB):
                                    nc.tensor.matmul(
                                        ps[:], mw2[:, hs, ob * P:(ob + 1) * P],
                                        hid[:, hs, t4 * TCH:(t4 + 1) * TCH],
                                        start=(hs == 0), stop=(hs == HSUB - 1))
                                nc.vector.tensor_scalar_add(
                                    ot[:, oo], ps[:], mb2_sb[:, b, ob:ob + 1])
                            nc.gpsimd.dma_start(
                                outp.ap()[b, obq * 256:(obq + 1) * 256,
                                          t4 * TCH:(t4 + 1) * TCH].rearrange(
                                              "(o p) t -> p o t", p=P),
                                ot[:])
                    if b + 1 < NB:
                        mw2n = mwp.tile([P, HSUB, D], F32R, tag="mw2")
                        nc.sync.dma_start(
                            mw2n[:],
                            mW2d.ap()[b + 1].rearrange("s c g h -> c g s h"))
                        mw1, mw2 = mw1n, mw2n

    nc.compile()
    return nc


def _get_nc():
    global _CACHED_NC
    if _CACHED_NC is None:
        _CACHED_NC = _build()
    return _CACHED_NC


def kernel(x, mask, router_w, router_b, W1, b1, W2, b2, _trace=False):
    x = np.asarray(x, np.float32)
    mask = np.asarray(mask, np.float32)
    router_w = np.asarray(router_w, np.float32)
    router_b = np.asarray(router_b, np.float32)
    W1 = np.asarray(W1, np.float32)
    b1 = np.asarray(b1, np.float32)
    W2 = np.asarray(W2, np.float32)
    b2 = np.asarray(b2, np.float32)

    nc = _get_nc()

    # host-side layout prep (sharding): transposes only, no reductions
    xTb_all = np.ascontiguousarray(
        x.transpose(0, 2, 1)).astype(ml_dtypes.bfloat16)       # [B, D, L]
    W1T_all = W1.transpose(0, 2, 1)                            # [E, D, H]
    W2T_all = W2.transpose(0, 2, 1)                            # [E, H, D]
    rwT = np.ascontiguousarray(router_w.T).astype(ml_dtypes.bfloat16)  # [D, E]
    rbc = np.ascontiguousarray(router_b.reshape(E, 1))
    b1T_full = np.ascontiguousarray(b1.T)                      # [H, E]
    b2T = np.ascontiguousarray(b2.T)                           # [D, E]

    pp = np.arange(P)
    gmaskc = (pp[:, None] // 8 == np.arange(16)[None, :]).astype(np.float32)
    emapc = (pp[:, None] % 8 == np.arange(E)[None, :]).astype(np.float32)

    in_maps = []
    for c in range(8):
        g, r = c // 4, c % 4
        hs = slice(r * HS, (r + 1) * HS)
        own = np.zeros((NB, 1), np.float32)
        own[r, 0] = 1.0
        msh = mask[g * NB:(g + 1) * NB]                        # [NB, L]
        # W1b[sb, dc, (g,e), h] = W1T_sh[e, sb*128+dc*16+g, h]
        W1sh = W1T_all[:, :, hs]                               # [E, D, HS]
        W1b = np.ascontiguousarray(
            W1sh.reshape(E, DS, 8, 16, HS).transpose(1, 2, 3, 0, 4).reshape(
                DS, 8, P, HS)).astype(np.float32)
        W2sh = W2T_all[:, hs, :]                               # [E, HS, D]
        W2b = np.ascontiguousarray(
            W2sh.reshape(E, HSUB, 8, 16, D).transpose(1, 2, 3, 0, 4).reshape(
                HSUB, 8, P, D)).astype(np.float32)
        in_maps.append({
            "xTb": xTb_all[g * NB:(g + 1) * NB],
            "maskT": np.ascontiguousarray(msh.T),
            "maskR8": np.ascontiguousarray(
                np.broadcast_to(msh[:, None, :], (NB, E, L))),
            "rwT": rwT,
            "rb": rbc,
            "W1b": W1b,
            "W2b": W2b,
            "b1T": np.ascontiguousarray(b1T_full[hs]),
            "b2T": b2T,
            "ownc": own,
            "gmaskc": gmaskc,
            "emapc": emapc,
        })

    res = run_bass_kernel_spmd(nc, in_maps, core_ids=list(range(8)),
                               trace=_trace)

    out = np.empty((B, L, D), np.float32)
    for g in range(2):
        acc = res.results[g * 4]["outp"].copy()
        for r in range(1, 4):
            acc += res.results[g * 4 + r]["outp"]
        for j in range(NB):
            out[g * NB + j] = acc[j].T
    if _trace:
        return out, res
    return out
